# revision 1
# baseline (speedup 1.0000x reference)
"""KimiSparseMoE Trainium2 kernel (8 NeuronCores, token-sharded).

Structure exploited (provable from the reference algorithm, verified
numerically): the group-limited top-k with the scatter(...,k,1) quirk can
only ever route to experts {0, 1, 2, 8, 16, 24}:
  - each token picks its top-2 groups (of 4); only expert 8g (column 0)
    of a selected group keeps a positive masked score,
  - the remaining top-4 slots are filled by zero-score ties, which
    jax.lax.top_k resolves to the smallest indices: {1,2} if group 0 was
    selected else {0,1}.
So experts 0 and 1 serve every token, expert 2 serves tokens that picked
group 0, and experts 8/16/24 serve tokens that picked groups 1/2/3; the
combine weights are the sigmoid scores of those experts renormalized.

Kernel layout: tokens are sharded 8 ways (128 per core). Each core
computes the router closed-form on-device, then runs the 6 hot experts +
the shared expert densely over its 128 tokens, with per-(token,expert)
coefficients folded into the swiglu output, accumulating all down
projections into one PSUM region. No inter-core communication; the host
only slices/packs inputs and concatenates the 8 disjoint output shards.
"""

import numpy as np

import concourse.bass as bass
import concourse.mybir as mybir
from concourse.tile import TileContext
from concourse.masks import make_identity
from concourse.bass_utils import run_bass_kernel_spmd

F32 = mybir.dt.float32
AX = mybir.AxisListType.X
ALU = mybir.AluOpType
ACT = mybir.ActivationFunctionType

N_CORES = 8
T, D, E, DFF = 1024, 2048, 32, 1024
TPC = T // N_CORES          # tokens per core
KD = D // 128               # 16 contraction tiles over D
KF = DFF // 128             # 8 contraction tiles over DFF
HOT = [0, 1, 2, 8, 16, 24]  # the only experts the router can select
NF = 1 + len(HOT)           # shared expert first, then hot experts
SCALING = 2.5
WDC = 1024                  # down-proj D-columns per wd piece

# matmul dtype mode for the expert FFNs: "bf16", "f32r", or "f32"
MODE = "bf16"

_MAX_WAITS = 1  # this container's walrus accepts one sem-wait per instruction


def _split_sync_waits(nc):
    for fn in nc.m.functions:
        for blk in fn.blocks:
            old = list(blk.instructions)
            new = []
            changed = False
            for ins in old:
                si = ins.sync_info
                if si is not None and len(si.on_wait) > _MAX_WAITS:
                    waits = list(si.on_wait)
                    keep, rest = waits[:_MAX_WAITS], waits[_MAX_WAITS:]
                    for i in range(0, len(rest), _MAX_WAITS):
                        nop = mybir.InstNoOp(
                            name=nc.get_next_instruction_name(),
                            engine=ins.engine,
                            sync_info=mybir.SyncInfo(
                                on_wait=rest[i : i + _MAX_WAITS], on_update=[]
                            ),
                            bass_nofuse=True,
                        )
                        new.append(nop)
                        changed = True
                    si.on_wait = keep
                new.append(ins)
            if changed:
                blk.instructions = new


def _mode_params(mode):
    if mode == "bf16":
        return mybir.dt.bfloat16, 512
    if mode == "f32r":
        return mybir.dt.float32r, 256
    return mybir.dt.float32, 256


def build(mode=MODE):
    """Build the SPMD Bass program (identical on all 8 cores)."""
    wdt, CH = _mode_params(mode)
    # transpose path dtype: float32r can't be memset/transposed directly
    tdt = F32 if wdt == mybir.dt.float32r else wdt
    n_ch = DFF // CH

    nc = bass.Bass("TRN2", target_bir_lowering=False, debug=False, num_devices=N_CORES)

    xt_d = nc.dram_tensor("xt", [128, KD * TPC], F32, kind="ExternalInput")
    gwt_d = nc.dram_tensor("gwt", [128, KD * E], F32, kind="ExternalInput")
    biasr_d = nc.dram_tensor("biasr", [TPC, E], F32, kind="ExternalInput")
    wg_d = nc.dram_tensor("wg", [NF * n_ch, 128, KD * CH], wdt, kind="ExternalInput")
    wu_d = nc.dram_tensor("wu", [NF * n_ch, 128, KD * CH], wdt, kind="ExternalInput")
    wd_d = nc.dram_tensor("wd", [NF * 2, 128, KF * WDC], wdt, kind="ExternalInput")
    out_d = nc.dram_tensor("out", [TPC, D], F32, kind="ExternalOutput")


    wbufs = 2 if mybir.dt.np(wdt) == np.float32 else 3
    with TileContext(nc) as tc:
        with (
            tc.sbuf_pool(name="const", bufs=1) as cpool,
            tc.sbuf_pool(name="rt", bufs=1) as rt,
            tc.sbuf_pool(name="wgp", bufs=wbufs) as wgp,
            tc.sbuf_pool(name="wup", bufs=wbufs) as wup,
            tc.sbuf_pool(name="wdp", bufs=wbufs) as wdp,
            tc.sbuf_pool(name="hp", bufs=2) as hp,
            tc.sbuf_pool(name="silup", bufs=2) as silup,
            tc.sbuf_pool(name="hTp", bufs=2) as hTp,
            tc.psum_pool(name="gup", bufs=1) as gup,
            tc.psum_pool(name="tpp", bufs=2) as tpp,
            tc.psum_pool(name="outp", bufs=1) as outp,
        ):
            xt_sb = cpool.tile([128, KD * TPC], F32)
            nc.sync.dma_start(xt_sb, xt_d[:, :])
            if wdt != F32:
                xtb_sb = cpool.tile([128, KD * TPC], wdt)
                nc.vector.tensor_copy(xtb_sb, xt_sb)
            else:
                xtb_sb = xt_sb
            gwt_sb = cpool.tile([128, KD * E], F32)
            nc.sync.dma_start(gwt_sb, gwt_d[:, :])
            biasr_sb = cpool.tile([TPC, E], F32)
            nc.sync.dma_start(biasr_sb, biasr_d[:, :])
            identity = cpool.tile([128, 128], tdt)
            make_identity(nc, identity)

            # ---- router ----
            gates_ps = gup.tile([TPC, E], F32, tag="g")
            for k in range(KD):
                nc.tensor.matmul(
                    gates_ps,
                    lhsT=xt_sb[:, k * TPC : (k + 1) * TPC],
                    rhs=gwt_sb[:, k * E : (k + 1) * E],
                    start=(k == 0),
                    stop=(k == KD - 1),
                )
            s_sb = rt.tile([TPC, E], F32)
            nc.scalar.activation(s_sb, gates_ps, ACT.Sigmoid)
            sb_sb = rt.tile([TPC, E], F32)
            nc.vector.tensor_add(sb_sb, s_sb, biasr_sb)

            gs = rt.tile([TPC, 4], F32)
            for g in range(4):
                grp = sb_sb[:, 8 * g : 8 * g + 8]
                m1 = rt.tile([TPC, 1], F32, tag="m1")
                nc.vector.reduce_max(m1, grp, AX)
                eq = rt.tile([TPC, 8], F32, tag="eq")
                nc.vector.tensor_scalar(eq, grp, m1, None, ALU.is_equal)
                t2 = rt.tile([TPC, 8], F32, tag="t2")
                nc.vector.scalar_tensor_tensor(t2, eq, -1e30, grp, ALU.mult, ALU.add)
                m2 = rt.tile([TPC, 1], F32, tag="m2")
                nc.vector.reduce_max(m2, t2, AX)
                nc.vector.tensor_tensor(gs[:, g : g + 1], m1, m2, ALU.add)

            g1 = rt.tile([TPC, 1], F32)
            eq1 = rt.tile([TPC, 4], F32)
            gsm = rt.tile([TPC, 4], F32)
            g2 = rt.tile([TPC, 1], F32)
            eq2 = rt.tile([TPC, 4], F32)
            gmask = rt.tile([TPC, 4], F32)
            nc.vector.reduce_max(g1, gs, AX)
            nc.vector.tensor_scalar(eq1, gs, g1, None, ALU.is_equal)
            nc.vector.scalar_tensor_tensor(gsm, eq1, -1e30, gs, ALU.mult, ALU.add)
            nc.vector.reduce_max(g2, gsm, AX)
            nc.vector.tensor_scalar(eq2, gsm, g2, None, ALU.is_equal)
            nc.vector.tensor_add(gmask, eq1, eq2)

            hs = rt.tile([TPC, 6], F32)
            nc.vector.tensor_copy(hs[:, 0:3], s_sb[:, 0:3])
            nc.vector.tensor_copy(hs[:, 3:4], s_sb[:, 8:9])
            nc.vector.tensor_copy(hs[:, 4:5], s_sb[:, 16:17])
            nc.vector.tensor_copy(hs[:, 5:6], s_sb[:, 24:25])
            nc.vector.tensor_tensor(hs[:, 2:6], hs[:, 2:6], gmask, ALU.mult)
            denom = rt.tile([TPC, 1], F32)
            nc.vector.reduce_sum(denom, hs, AX)
            rec = rt.tile([TPC, 1], F32)
            nc.vector.reciprocal(rec, denom)
            coeff = rt.tile([TPC, 6], F32)
            nc.vector.tensor_scalar(coeff, hs, rec, SCALING, ALU.mult, ALU.mult)

            # ---- expert FFNs (shared first, then the 6 hot experts) ----
            out_ps = outp.tile([TPC, D], F32)
            out_sb = cpool.tile([TPC, D], F32)
            for f in range(NF):
                hT = hTp.tile([128, DFF], wdt, tag="hT")
                for ch in range(n_ch):
                    wgc = wgp.tile([128, KD * CH], wdt, tag="wg")
                    nc.sync.dma_start(wgc, wg_d[f * n_ch + ch])
                    wuc = wup.tile([128, KD * CH], wdt, tag="wu")
                    nc.sync.dma_start(wuc, wu_d[f * n_ch + ch])

                    G = gup.tile([TPC, CH], F32, tag="g")
                    for k in range(KD):
                        nc.tensor.matmul(
                            G,
                            lhsT=xtb_sb[:, k * TPC : (k + 1) * TPC],
                            rhs=wgc[:, k * CH : (k + 1) * CH],
                            start=(k == 0),
                            stop=(k == KD - 1),
                        )
                    U = gup.tile([TPC, CH], F32, tag="u")
                    for k in range(KD):
                        nc.tensor.matmul(
                            U,
                            lhsT=xtb_sb[:, k * TPC : (k + 1) * TPC],
                            rhs=wuc[:, k * CH : (k + 1) * CH],
                            start=(k == 0),
                            stop=(k == KD - 1),
                        )
                    silu_t = silup.tile([TPC, CH], F32, tag="silu")
                    nc.scalar.activation(silu_t, G, ACT.Silu)
                    h_t = hp.tile([TPC, CH], tdt, tag="h")
                    if f == 0:
                        nc.vector.tensor_tensor(h_t, silu_t, U, ALU.mult)
                    else:
                        nc.vector.scalar_tensor_tensor(
                            h_t, silu_t, coeff[:, f - 1 : f], U, ALU.mult, ALU.mult
                        )
                    for j2 in range(CH // 128):
                        tp = tpp.tile([128, 128], tdt, tag="tp")
                        nc.tensor.transpose(
                            tp, h_t[:, j2 * 128 : (j2 + 1) * 128], identity
                        )
                        kk = ch * (CH // 128) + j2
                        nc.vector.tensor_copy(hT[:, kk * 128 : (kk + 1) * 128], tp)

                for piece in range(2):
                    wdc_t = wdp.tile([128, KF * WDC], wdt, tag="wd")
                    nc.sync.dma_start(wdc_t, wd_d[f * 2 + piece])
                    for j2 in range(WDC // 512):
                        col = piece * WDC + j2 * 512
                        for k in range(KF):
                            nc.tensor.matmul(
                                out_ps[:, col : col + 512],
                                lhsT=hT[:, k * 128 : (k + 1) * 128],
                                rhs=wdc_t[:, k * WDC + j2 * 512 : k * WDC + (j2 + 1) * 512],
                                start=(f == 0 and k == 0),
                                stop=(f == NF - 1 and k == KF - 1),
                            )
                    if f == NF - 1:
                        pc = slice(piece * WDC, (piece + 1) * WDC)
                        nc.vector.tensor_copy(out_sb[:, pc], out_ps[:, pc])
                        nc.sync.dma_start(out_d[:, pc], out_sb[:, pc])

    _split_sync_waits(nc)
    return nc


def _pack_sbuf16(mat_t, cols, np_dt):
    """[D_rows, cols] (row-major, D_rows = 128*K) -> SBUF image [128, K*cols]."""
    rows = mat_t.shape[0]
    k = rows // 128
    return (
        np.ascontiguousarray(mat_t)
        .reshape(k, 128, cols)
        .transpose(1, 0, 2)
        .reshape(128, k * cols)
        .astype(np_dt, copy=False)
    )


def _pack_inputs(x, gate_w, bias, Wg, Wu, Wd, sWg, sWu, sWd, mode=MODE):
    x = np.asarray(x, np.float32)
    gate_w = np.asarray(gate_w, np.float32)
    bias = np.asarray(bias, np.float32)
    Wg, Wu, Wd = (np.asarray(a, np.float32) for a in (Wg, Wu, Wd))
    sWg, sWu, sWd = (np.asarray(a, np.float32) for a in (sWg, sWu, sWd))
    wdt, CH = _mode_params(mode)
    np_wdt = mybir.dt.np(wdt)
    n_ch = DFF // CH

    ffn = [(sWg, sWu, sWd)] + [(Wg[e], Wu[e], Wd[e]) for e in HOT]

    wg_all = np.empty((NF * n_ch, 128, KD * CH), np_wdt)
    wu_all = np.empty((NF * n_ch, 128, KD * CH), np_wdt)
    wd_all = np.empty((NF * 2, 128, KF * WDC), np_wdt)
    for f, (wgf, wuf, wdf) in enumerate(ffn):
        wgT = np.ascontiguousarray(wgf.T)  # [D, DFF]
        wuT = np.ascontiguousarray(wuf.T)
        wdT = np.ascontiguousarray(wdf.T)  # [DFF, D]
        for ch in range(n_ch):
            wg_all[f * n_ch + ch] = _pack_sbuf16(
                wgT[:, ch * CH : (ch + 1) * CH], CH, np_wdt
            )
            wu_all[f * n_ch + ch] = _pack_sbuf16(
                wuT[:, ch * CH : (ch + 1) * CH], CH, np_wdt
            )
        for piece in range(2):
            wd_all[f * 2 + piece] = _pack_sbuf16(
                wdT[:, piece * WDC : (piece + 1) * WDC], WDC, np_wdt
            )

    gwt = _pack_sbuf16(np.ascontiguousarray(gate_w.T), E, np.float32)
    biasr = np.broadcast_to(np.asarray(bias, np.float32), (TPC, E)).copy()

    in_maps = []
    for c in range(N_CORES):
        xc = np.asarray(x[c * TPC : (c + 1) * TPC], np.float32)
        xt = _pack_sbuf16(np.ascontiguousarray(xc.T), TPC, np.float32)
        m = {
            "xt": xt,
            "gwt": gwt,
            "biasr": biasr,
            "wg": wg_all,
            "wu": wu_all,
            "wd": wd_all,
        }
        in_maps.append(m)
    return in_maps


def run(inputs, mode=MODE, trace=False):
    nc = build(mode)
    in_maps = _pack_inputs(**inputs, mode=mode)
    res = run_bass_kernel_spmd(
        nc, in_maps, core_ids=list(range(N_CORES)), trace=trace
    )
    out = np.concatenate([res.results[c]["out"] for c in range(N_CORES)], axis=0)
    return out, res


def kernel(**inputs):
    out, _ = run(inputs, mode=MODE, trace=False)
    return out

